# revision 1
# baseline (speedup 1.0000x reference)
"""Multi-head attention (b=16, l=1025, d=768, H=12) on 8 TRN2 NeuronCores.

Sharding: data-parallel over batch - 2 batch elements per core, no
collectives.

Per-core kernel (per batch element), layouts transposed so the sequence
dim is the matmul free dim:
  1. QK^T = (Wqk stationary) @ X^T            -> [1536, L]  (bf16)
  2. V    = (X^T blocks stationary) @ Wv      -> [L, 768] stored per-head
     as [L, 12*(64+1)] with a ones column per head (so the PV matmul's
     65th output row is the softmax denominator), padded to 843 cols so
     PV weight loads are 128 wide (enables FWL).
  3. Per head pair g (heads 2g / 2g+1 share the PE array as row groups
     0-63 / 64-127), per query chunk c in {[0:512], [512:1024]}, per
     key block j: one 2-bank psum tile holds both heads' S^T; one ACT
     instruction computes P^T = exp(S^T/8) (no max subtraction - scores
     are O(1)); PV accumulates O_aug^T[65, i] two key blocks behind the
     scores so the in-order PE never waits on an exp it just issued.
     O^T = O_aug^T[0:64] * recip(O_aug^T[64]) via DVE + gpsimd
     partition_broadcast.
  4. Y^T = (Wo stationary) @ O^T + bo         -> [768, L] fp32 -> DRAM

Query 1024 (the l=1025 straggler) is reconstructed on the host from the
exported bf16 K^T / V, so every device loop is a power of two. Element
1's projections are interleaved (via generators that yield every ~2
matmuls) into element 0's attention as PE filler while ACT drains the
exps, and element 0's output projection into element 1's attention.

Host side: permute Wqkv from interleaved-head to head-contiguous order,
transpose inputs/outputs, cast to bf16.
"""

import contextlib

import numpy as np
import ml_dtypes

import concourse.bass as bass
import concourse.bacc as bacc
import concourse.mybir as mybir
import concourse.tile as tile
from concourse.bass_utils import run_bass_kernel_spmd

N_CORES = 8
B = 16
L = 1025
D = 768
H = 12
DH = 64
BPC = B // N_CORES
KT = D // 128   # 6 contraction tiles
JT = (L + 127) // 128  # 9 j-tiles; last has 1 row
SCALE = 1.0 / np.sqrt(DH)

BF16 = mybir.dt.bfloat16
F32 = mybir.dt.float32
EXP = mybir.ActivationFunctionType.Exp
MULT = mybir.AluOpType.mult
ADD = mybir.AluOpType.add

_CACHE = {}


def _build():
    nc = bacc.Bacc("TRN2", target_bir_lowering=False, debug=False,
                   num_devices=N_CORES)
    xT = nc.dram_tensor("xT", [BPC, D, L], BF16, kind="ExternalInput")
    w_qk = nc.dram_tensor("w_qk", [D, 2 * D], BF16, kind="ExternalInput")
    w_v = nc.dram_tensor("w_v", [D, D], BF16, kind="ExternalInput")
    w_o = nc.dram_tensor("w_o", [D, D], BF16, kind="ExternalInput")
    b_qk = nc.dram_tensor("b_qk", [2 * D, 1], F32, kind="ExternalInput")
    b_v = nc.dram_tensor("b_v", [1, D], F32, kind="ExternalInput")
    b_o = nc.dram_tensor("b_o", [D, 1], F32, kind="ExternalInput")
    yT = nc.dram_tensor("yT", [BPC, D, L], F32, kind="ExternalOutput")
    kTo = nc.dram_tensor("kTo", [BPC, D, L], BF16, kind="ExternalOutput")
    vo = nc.dram_tensor("vo", [BPC, JT, 128, H * (DH + 1)], BF16,
                        kind="ExternalOutput")

    with tile.TileContext(nc) as tc:
        _emit(nc, tc, xT, w_qk, w_v, w_o, b_qk, b_v, b_o, yT, kTo, vo)
    nc.compile()
    return nc


def _ap(t, poff, pcount, foff, fdims):
    """AP on tile t at partition offset poff (count pcount), free offset
    foff with free dims [(step, count), ...]."""
    base = t[:]
    pstep = base.ap[0][0]
    return bass.AP(tensor=base.tensor,
                   offset=base.offset + poff * pstep + foff,
                   ap=[[pstep, pcount]] + [list(d) for d in fdims])


def _emit(nc, tc, xT, w_qk, w_v, w_o, b_qk, b_v, b_o, yT, kTo, vo):
    ctx = contextlib.ExitStack()
    with ctx:
        consts = ctx.enter_context(tc.tile_pool(name="consts", bufs=1))
        xpool = ctx.enter_context(tc.tile_pool(name="xpool", bufs=1))
        qkpool = ctx.enter_context(tc.tile_pool(name="qkpool", bufs=2))
        vpool = ctx.enter_context(tc.tile_pool(name="vpool", bufs=2))
        otpool = ctx.enter_context(tc.tile_pool(name="otpool", bufs=2))
        ytpool = ctx.enter_context(tc.tile_pool(name="ytpool", bufs=2))
        ptpool = ctx.enter_context(tc.tile_pool(name="ptpool", bufs=6))
        smpool = ctx.enter_context(tc.tile_pool(name="smpool", bufs=3))
        # PSUM: big 2x[128,1024]=4 banks, small 1, acc 3 -> 8
        bigp = ctx.enter_context(tc.tile_pool(name="bigp", bufs=2, space="PSUM"))
        smallp = ctx.enter_context(tc.tile_pool(name="smallp", bufs=2, space="PSUM"))
        accp = ctx.enter_context(tc.tile_pool(name="accp", bufs=2, space="PSUM"))

        # ---- constants (xt emitted first by the schedule; wo last) ----
        wqk_t = [consts.tile([128, 2 * D], BF16, name=f"wqk{k}") for k in range(KT)]
        wv_t = [consts.tile([128, D], BF16, name=f"wv{k}") for k in range(KT)]
        wo_t = [consts.tile([128, D], BF16, name=f"wo{k}") for k in range(KT)]
        bqk_t = [consts.tile([128, 1], F32, name=f"bqk{m}") for m in range(2 * KT)]
        bo_t = [consts.tile([128, 1], F32, name=f"bo{m}") for m in range(KT)]
        bv_bc = consts.tile([128, D], F32, name="bvbc")

        xt = {}
        qkT = {}
        vt = {}
        oT = {}

        def load_x(e):
            xt[e] = [xpool.tile([128, L], BF16, tag=f"xt{k}", name=f"xt{e}_{k}")
                     for k in range(KT)]
            for k in range(KT):
                nc.sync.dma_start(out=xt[e][k][:],
                                  in_=xT[e, k * 128:(k + 1) * 128, :])

        def v_proj(e, jlist):
            """V[j,:] for j-tiles in jlist; layout [jlen, 12*(64+1)]."""
            if e not in vt:
                vt[e] = [vpool.tile([128, H * (DH + 1) + 63], BF16, tag=f"vt{j}",
                                    name=f"vt{e}_{j}") for j in range(JT)]
            for j in jlist:
                jlen = min(128, L - j * 128)
                nc.vector.memset(
                    _ap(vt[e][j], 0, 128, DH, [[DH + 1, H], [1, 1]]), 1.0)
                nc.vector.memset(
                    _ap(vt[e][j], 0, 128, H * (DH + 1), [[1, 63]]), 0.0)
                ps = bigp.tile([128, 1024], F32, tag="big", name=f"vps{e}_{j}")
                for k in range(KT):
                    nc.tensor.matmul(ps[:jlen, 0:512],
                                     xt[e][k][:, j * 128:j * 128 + jlen],
                                     wv_t[k][:, 0:512],
                                     start=(k == 0), stop=(k == KT - 1))
                for k in range(KT):
                    nc.tensor.matmul(ps[:jlen, 512:768],
                                     xt[e][k][:, j * 128:j * 128 + jlen],
                                     wv_t[k][:, 512:768],
                                     start=(k == 0), stop=(k == KT - 1))
                dst = _ap(vt[e][j], 0, jlen, 0, [[DH + 1, H], [1, DH]])
                src = _ap(ps, 0, jlen, 0, [[DH, H], [1, DH]])
                bia = _ap(bv_bc, 0, jlen, 0, [[DH, H], [1, DH]])
                nc.vector.tensor_tensor(out=dst, in0=src, in1=bia, op=ADD)
                nc.sync.dma_start(out=vo[e, j], in_=vt[e][j][:, 0:H * (DH + 1)])

        def qk_unit(e, m):
            """One QK^T m-tile: big psum (c0+c1), small straggler col."""
            if e not in qkT:
                qkT[e] = [qkpool.tile([128, L], BF16, tag=f"qkT{t}",
                                      name=f"qkT{e}_{t}") for t in range(2 * KT)]
            ps = bigp.tile([128, 1024], F32, tag="big", name=f"qkps{e}_{m}")
            for k in range(KT):
                nc.tensor.matmul(ps[:, 0:512],
                                 wqk_t[k][:, m * 128:(m + 1) * 128],
                                 xt[e][k][:, 0:512],
                                 start=(k == 0), stop=(k == KT - 1))
            for k in range(KT):
                nc.tensor.matmul(ps[:, 512:1024],
                                 wqk_t[k][:, m * 128:(m + 1) * 128],
                                 xt[e][k][:, 512:1024],
                                 start=(k == 0), stop=(k == KT - 1))
            nc.vector.tensor_scalar_add(qkT[e][m][:, 0:1024], ps[:, 0:1024],
                                        bqk_t[m][:])
            if m >= KT:
                sg = smallp.tile([128, 512], F32, tag="small",
                                 name=f"qksg{e}_{m}")
                for k in range(KT):
                    nc.tensor.matmul(sg[:, 0:1],
                                     wqk_t[k][:, m * 128:(m + 1) * 128],
                                     xt[e][k][:, 1024:1025],
                                     start=(k == 0), stop=(k == KT - 1))
                nc.vector.tensor_scalar_add(qkT[e][m][:, 1024:1025],
                                            sg[:, 0:1], bqk_t[m][:])
                nc.sync.dma_start(out=kTo[e, (m - KT) * 128:(m - KT + 1) * 128, :],
                                  in_=qkT[e][m][:])

        def small_chunk(name, nmm, mms, dve):
            """One projection chunk through the 1-bank small psum pool."""
            ps = smallp.tile([128, 512], F32, tag="small", name=name)
            for i in range(nmm):
                mms(ps, i)
                if i % 2 == 1:
                    yield
            dve(ps)

        def v_unit_gen(e, j):
            if e not in vt:
                vt[e] = [vpool.tile([128, H * (DH + 1) + 63], BF16, tag=f"vt{t}",
                                    name=f"vt{e}_{t}") for t in range(JT)]
            jlen = min(128, L - j * 128)
            nc.vector.memset(
                _ap(vt[e][j], 0, 128, DH, [[DH + 1, H], [1, 1]]), 1.0)
            nc.vector.memset(
                _ap(vt[e][j], 0, 128, H * (DH + 1), [[1, 63]]), 0.0)
            for c, (c0, nh) in enumerate(((0, 8), (512, 4))):
                def mms(ps, k, c0=c0, clen=64 * nh):
                    nc.tensor.matmul(ps[:jlen, 0:clen],
                                     xt[e][k][:, j * 128:j * 128 + jlen],
                                     wv_t[k][:, c0:c0 + clen],
                                     start=(k == 0), stop=(k == KT - 1))
                def dve(ps, c0=c0, nh=nh):
                    dst = _ap(vt[e][j], 0, jlen, (c0 // 64) * (DH + 1),
                              [[DH + 1, nh], [1, DH]])
                    src = _ap(ps, 0, jlen, 0, [[DH, nh], [1, DH]])
                    bia = _ap(bv_bc, 0, jlen, c0, [[DH, nh], [1, DH]])
                    nc.vector.tensor_tensor(out=dst, in0=src, in1=bia, op=ADD)
                yield from small_chunk(f"vg{e}_{j}_{c}", KT, mms, dve)
            nc.sync.dma_start(out=vo[e, j], in_=vt[e][j][:, 0:H * (DH + 1)])

        def qk_unit_gen(e, m):
            if e not in qkT:
                qkT[e] = [qkpool.tile([128, L], BF16, tag=f"qkT{t}",
                                      name=f"qkT{e}_{t}") for t in range(2 * KT)]
            for c in range(2):
                def mms(ps, k, c=c):
                    nc.tensor.matmul(ps[:, 0:512],
                                     wqk_t[k][:, m * 128:(m + 1) * 128],
                                     xt[e][k][:, c * 512:c * 512 + 512],
                                     start=(k == 0), stop=(k == KT - 1))
                def dve(ps, c=c):
                    nc.vector.tensor_scalar_add(
                        qkT[e][m][:, c * 512:c * 512 + 512],
                        ps[:, 0:512], bqk_t[m][:])
                yield from small_chunk(f"qg{e}_{m}_{c}", KT, mms, dve)
            if m >= KT:
                def mms(ps, k):
                    nc.tensor.matmul(ps[:, 0:1],
                                     wqk_t[k][:, m * 128:(m + 1) * 128],
                                     xt[e][k][:, 1024:1025],
                                     start=(k == 0), stop=(k == KT - 1))
                def dve(ps):
                    nc.vector.tensor_scalar_add(qkT[e][m][:, 1024:1025],
                                                ps[:, 0:1], bqk_t[m][:])
                yield from small_chunk(f"qgs{e}_{m}", KT, mms, dve)
                nc.sync.dma_start(
                    out=kTo[e, (m - KT) * 128:(m - KT + 1) * 128, :],
                    in_=qkT[e][m][:])

        def out_unit_gen(e, m):
            yt = ytpool.tile([128, L], F32, tag="yt", name=f"yt{e}_{m}")
            for c in range(2):
                def mms(ps, k, c=c):
                    nc.tensor.matmul(ps[:, 0:512],
                                     wo_t[k][:, m * 128:(m + 1) * 128],
                                     oT[e][k][:, c * 512:c * 512 + 512],
                                     start=(k == 0), stop=(k == KT - 1))
                def dve(ps, c=c):
                    nc.vector.tensor_scalar_add(yt[:, c * 512:c * 512 + 512],
                                                ps[:, 0:512], bo_t[m][:])
                yield from small_chunk(f"og{e}_{m}_{c}", KT, mms, dve)
                nc.sync.dma_start(
                    out=yT[e, m * 128:(m + 1) * 128, c * 512:c * 512 + 512],
                    in_=yt[:, c * 512:c * 512 + 512])

        def load_x_gen(e):
            load_x(e)
            yield

        class Fill:
            def __init__(self, gens):
                self.gens = list(gens)

            def pull(self, n=1):
                while n > 0 and self.gens:
                    try:
                        next(self.gens[0])
                        n -= 1
                    except StopIteration:
                        self.gens.pop(0)

            def finish(self, k):
                """Exhaust the first k remaining generators."""
                for gen in self.gens[:k]:
                    for _ in gen:
                        pass
                self.gens = self.gens[k:]

            def flush(self):
                self.finish(len(self.gens))

        def attention(e, g, fill=None, stride=1):
            """Head pair g: heads 2g (partitions 0-63), 2g+1 (64-127)."""
            fill = fill or Fill([])
            if e not in oT:
                oT[e] = [otpool.tile([128, L], BF16, tag=f"oT{t}",
                                     name=f"oT{e}_{t}") for t in range(KT)]
            kt_q, kt_k = qkT[e][g], qkT[e][KT + g]
            for (i0, ilen) in ((0, 512), (512, 512)):
                oacc = [accp.tile([128, 512], F32, tag="acc",
                                  name=f"oacc{e}_{g}_{i0}_{u}") for u in range(2)]
                # Per key-block j one big tile holds u0 scores (cols
                # 0:512) and u1 scores (cols 512:1024); the two K=64 mms are
                # adjacent instructions in disjoint PE row groups (0-63 /
                # 64-127) so they overlap on hardware. PV runs 2 key-blocks
                # behind scores (2 big slots); filler plugs residual stalls.
                pts = []

                def pv(j):
                    # lhsT padded to 128 cols (past the head's 65) so the
                    # compiler enables FWL; out rows 65-127 are garbage.
                    pt = pts[j]
                    for u in range(2):
                        h = 2 * g + u
                        nc.tensor.matmul(
                            oacc[u][:128, :ilen],
                            vt[e][j][:, h * (DH + 1):h * (DH + 1) + 128],
                            pt[:, u * 512:u * 512 + ilen],
                            start=(j == 0), stop=False)

                for j in range(8):
                    if j >= 2:
                        pv(j - 2)
                    sps = bigp.tile([128, 1024], F32, tag="big",
                                    name=f"sps{e}_{g}_{i0}_{j}")
                    for u in range(2):
                        nc.tensor.matmul(
                            sps[:128, u * 512:u * 512 + ilen],
                            kt_k[u * 64:(u + 1) * 64, j * 128:(j + 1) * 128],
                            kt_q[u * 64:(u + 1) * 64, i0:i0 + ilen],
                            start=True, stop=True)
                    pt = ptpool.tile([128, 1024], BF16, tag="pt",
                                     name=f"pt{e}_{g}_{i0}_{j}")
                    nc.scalar.activation(pt[:, :], sps[:, :], EXP,
                                         bias=0.0, scale=float(SCALE))
                    pts.append(pt)
                    if j % stride == 0:
                        fill.pull(1)
                pv(6)
                fill.pull(1)
                pv(7)
                fill.pull(1)
                # j8 (jlen=1): u0 in cols 0:512, u1 in cols 512:1024,
                # both at partition 0 so PV lhsT/rhs bases match
                sp8 = bigp.tile([128, 1024], F32, tag="big",
                                name=f"sp8{e}_{g}_{i0}")
                for u in range(2):
                    nc.tensor.matmul(
                        sp8[0:1, u * 512:u * 512 + ilen],
                        kt_k[u * 64:(u + 1) * 64, 1024:1025],
                        kt_q[u * 64:(u + 1) * 64, i0:i0 + ilen],
                        start=True, stop=True)
                pt8 = ptpool.tile([1, 1024], BF16, tag="pt8",
                                  name=f"pt8{e}_{g}_{i0}")
                nc.scalar.activation(pt8[:1, :], sp8[:1, :], EXP,
                                     bias=0.0, scale=float(SCALE))
                for u in range(2):
                    h = 2 * g + u
                    nc.tensor.matmul(
                        oacc[u][:128, :ilen],
                        vt[e][JT - 1][:1, h * (DH + 1):h * (DH + 1) + 128],
                        pt8[0:1, u * 512:u * 512 + ilen],
                        start=False, stop=True)
                fill.pull(1)
                # normalize
                for u in range(2):
                    rec1 = smpool.tile([1, 512], F32, tag="rec1",
                                       name=f"rec1{e}_{g}_{i0}_{u}")
                    nc.vector.reciprocal(rec1[:1, :ilen],
                                         oacc[u][DH:DH + 1, :ilen])
                    rec = smpool.tile([128, 512], F32, tag="rec",
                                      name=f"rec{e}_{g}_{i0}_{u}")
                    nc.gpsimd.partition_broadcast(rec[:DH, :ilen],
                                                  rec1[:1, :ilen])
                    nc.vector.tensor_tensor(
                        out=oT[e][g][u * 64:(u + 1) * 64, i0:i0 + ilen],
                        in0=oacc[u][:DH, :ilen], in1=rec[:DH, :ilen], op=MULT)
                fill.pull(1)

        # ---- schedule ----
        # warm the exp table during the input DMA shadow
        warm = smpool.tile([1, 512], F32, tag="rec1", name="warm")
        nc.vector.memset(warm[:1, 0:1], 0.0)
        nc.scalar.activation(warm[:1, 0:1], warm[:1, 0:1], EXP,
                             bias=0.0, scale=1.0)
        # interleave xt[k] / wv[k] so v_proj's k-th matmul can start as
        # soon as the k-th pair lands
        xt[0] = [xpool.tile([128, L], BF16, tag=f"xt{k}", name=f"xt0_{k}")
                 for k in range(KT)]
        for k in range(KT):
            nc.sync.dma_start(out=xt[0][k][:],
                              in_=xT[0, k * 128:(k + 1) * 128, :])
            nc.sync.dma_start(out=wv_t[k][:], in_=w_v[k * 128:(k + 1) * 128, :])
        bva = b_v[:]
        nc.sync.dma_start(out=bv_bc[:], in_=bass.AP(
            tensor=bva.tensor, offset=bva.offset,
            ap=[[0, 128], list(bva.ap[1])]))
        for k in range(KT):
            nc.sync.dma_start(out=wqk_t[k][:], in_=w_qk[k * 128:(k + 1) * 128, :])
        for m in range(2 * KT):
            nc.sync.dma_start(out=bqk_t[m][:], in_=b_qk[m * 128:(m + 1) * 128, :])
        for m in range(KT):
            nc.sync.dma_start(out=bo_t[m][:], in_=b_o[m * 128:(m + 1) * 128, :])
        for k in range(KT):
            nc.sync.dma_start(out=wo_t[k][:], in_=w_o[k * 128:(k + 1) * 128, :])
        v_proj(0, list(range(JT)))
        qk_unit(0, 0); qk_unit(0, KT)
        gens = []
        for g in range(1, KT):
            gens += [qk_unit_gen(0, g), qk_unit_gen(0, KT + g)]
        gens += [load_x_gen(1)]
        gens += [v_unit_gen(1, j) for j in range(JT)]
        gens += [qk_unit_gen(1, 0), qk_unit_gen(1, KT)]
        fill = Fill(gens)
        for g in range(KT):
            if g >= 1:
                # the pair's own QK tiles must be complete before its scores
                fill.finish(2)
            attention(0, g, fill, stride=2)
        fill.flush()
        gens = []
        for g in range(1, KT):
            gens += [qk_unit_gen(1, g), qk_unit_gen(1, KT + g)]
        gens += [out_unit_gen(0, m) for m in range(KT)]
        fill = Fill(gens)
        for g in range(KT):
            if g >= 1:
                fill.finish(2)
            attention(1, g, fill, stride=2)
        fill.flush()
        for m in range(KT):
            for _ in out_unit_gen(1, m):
                pass


def _prep_inputs(query, Wqkv, bqkv, Wo, bo):
    Wp = Wqkv.reshape(D, 3, DH, H).transpose(0, 1, 3, 2).reshape(D, 3 * D)
    bp = bqkv.reshape(3, DH, H).transpose(0, 2, 1).reshape(3 * D)
    w_qk = np.ascontiguousarray(Wp[:, :2 * D]).astype(ml_dtypes.bfloat16)
    w_v = np.ascontiguousarray(Wp[:, 2 * D:]).astype(ml_dtypes.bfloat16)
    w_o = np.ascontiguousarray(Wo).astype(ml_dtypes.bfloat16)
    b_qk = np.ascontiguousarray(bp[:2 * D]).astype(np.float32).reshape(2 * D, 1)
    b_v = np.ascontiguousarray(bp[2 * D:]).astype(np.float32).reshape(1, D)
    b_o = np.ascontiguousarray(bo).astype(np.float32).reshape(D, 1)

    in_maps = []
    for c in range(N_CORES):
        xc = query[c * BPC:(c + 1) * BPC]
        xTc = np.ascontiguousarray(xc.transpose(0, 2, 1)).astype(
            ml_dtypes.bfloat16)
        in_maps.append(dict(xT=xTc, w_qk=w_qk, w_v=w_v, w_o=w_o,
                            b_qk=b_qk, b_v=b_v, b_o=b_o))
    return in_maps


def kernel(query, Wqkv, bqkv, Wo, bo):
    query = np.asarray(query, dtype=np.float32)
    Wqkv = np.asarray(Wqkv, dtype=np.float32)
    bqkv = np.asarray(bqkv, dtype=np.float32)
    Wo = np.asarray(Wo, dtype=np.float32)
    bo = np.asarray(bo, dtype=np.float32)

    if "nc" not in _CACHE:
        _CACHE["nc"] = _build()
    nc = _CACHE["nc"]

    in_maps = _prep_inputs(query, Wqkv, bqkv, Wo, bo)
    res = run_bass_kernel_spmd(nc, in_maps, core_ids=list(range(N_CORES)))
    out = np.empty((B, L, D), dtype=np.float32)
    # The device computes queries 0..1023; query 1024 is reconstructed on
    # the host from the exported (bf16) K^T and V.
    Wp = Wqkv.reshape(D, 3, DH, H).transpose(0, 1, 3, 2).reshape(D, 3 * D)
    bp = bqkv.reshape(3, DH, H).transpose(0, 2, 1).reshape(3 * D)
    for c in range(N_CORES):
        r = res.results[c]
        out[c * BPC:(c + 1) * BPC] = r["yT"].transpose(0, 2, 1)
        kT = np.asarray(r["kTo"], dtype=np.float32)   # [BPC, 768, L]
        v = np.asarray(r["vo"], dtype=np.float32)     # [BPC, JT, 128, 780]
        for e in range(BPC):
            b = c * BPC + e
            qrow = query[b, L - 1] @ Wp[:, :D] + bp[:D]          # [768]
            vfull = v[e].reshape(JT * 128, H * (DH + 1))[:L]     # [L, 780]
            orow = np.empty(D, dtype=np.float32)
            for h in range(H):
                g, u = h // 2, h % 2
                kh = kT[e, g * 128 + u * 64:g * 128 + u * 64 + 64]  # [64,L]
                sh = (qrow[h * DH:(h + 1) * DH] @ kh) * SCALE
                ph = np.exp(sh - sh.max())
                vh = vfull[:, h * (DH + 1):h * (DH + 1) + DH]
                orow[h * DH:(h + 1) * DH] = (ph @ vh) / ph.sum()
            out[b, L - 1] = orow @ Wo + bo
    return out



# revision 2
# speedup vs baseline: 1.0943x; 1.0943x over previous
"""Multi-head attention (b=16, l=1025, d=768, H=12) on 8 TRN2 NeuronCores.

Sharding: data-parallel over batch - 2 batch elements per core, no
collectives.

Per-core kernel v2. Key ideas vs the v1 baseline:

1. Scores via fp8 DoubleRow matmuls: K is split into fp8 hi+lo planes
   (error-compensated), Q is single fp8 (one-sided). One DoubleRow matmul
   computes Khi^T.Qhi + Klo^T.Qhi = K^T.Qhi at 0.5 cycles/row - half the
   bf16 cost. The K bias is dropped entirely (softmax over keys is
   invariant to a per-query constant); the Q bias is folded into Qhi.
2. PV in O-orientation: out O[q,64] per (head, key-block, q-block) with
   lhsT = P^T slice [128 keys, 128 q], rhs = V [128 keys, 64]. Full
   contraction + full output partitions; the free dim is only 64, so PV
   costs half of the old O^T orientation. Softmax denominators come from
   separate N=1 matmuls against a ones vector (nearly free).
3. O is transposed back to O^T for the output projection with the DMA
   xbar transpose ([128,768] -> [128,6,128] in one instruction) - zero
   PE/ACT/DVE cost.
4. The l=1025 stragglers: query 1024 is fully handled on the host (from
   exported K/V); key 1024's rank-12 contribution to Y is added on the
   host from exported p8 = exp(s8) and denominators, where s8 is computed
   on device as [q,1]-oriented N=1 matmuls.

The schedule interleaves projection matmuls (element e+1's QKV, element
e's output proj) into the ACT-paced attention loop as PE filler, via
the same generator mechanism as v1.
"""

import contextlib

import numpy as np
import ml_dtypes

import concourse.bass as bass
import concourse.bacc as bacc
import concourse.mybir as mybir
import concourse.tile as tile
from concourse.bass_utils import run_bass_kernel_spmd

N_CORES = 8
B = 16
L = 1025
D = 768
H = 12
DH = 64
BPC = B // N_CORES
KT = D // 128   # 6 contraction tiles
NJ = 8          # full 128-key blocks; key 1024 handled via s8/p8
SCALE = 1.0 / np.sqrt(DH)
KLO = 1040      # Klo plane offset inside khilo tiles (16-aligned)

BF16 = mybir.dt.bfloat16
F32 = mybir.dt.float32
FP8 = mybir.dt.float8e4
NPF8 = ml_dtypes.float8_e4m3
EXP = mybir.ActivationFunctionType.Exp
MULT = mybir.AluOpType.mult
ADD = mybir.AluOpType.add
SUB = mybir.AluOpType.subtract
DR = mybir.MatmulPerfMode.DoubleRow

_CACHE = {}


def _ap(t, poff, pcount, foff, fdims):
    """AP on tile t at partition offset poff (count pcount), free offset
    foff with free dims [(step, count), ...]."""
    base = t[:]
    pstep = base.ap[0][0]
    return bass.AP(tensor=base.tensor,
                   offset=base.offset + poff * pstep + foff,
                   ap=[[pstep, pcount]] + [list(d) for d in fdims])


def _build():
    nc = bacc.Bacc("TRN2", target_bir_lowering=False, debug=False,
                   num_devices=N_CORES)
    xT = nc.dram_tensor("xT", [BPC, D, L], BF16, kind="ExternalInput")
    w_qk = nc.dram_tensor("w_qk", [D, 2 * D], BF16, kind="ExternalInput")
    w_v = nc.dram_tensor("w_v", [D, D], BF16, kind="ExternalInput")
    w_o = nc.dram_tensor("w_o", [D, D], BF16, kind="ExternalInput")
    b_q = nc.dram_tensor("b_q", [D, 1], F32, kind="ExternalInput")
    b_v = nc.dram_tensor("b_v", [1, D], F32, kind="ExternalInput")
    b_o = nc.dram_tensor("b_o", [D, 1], F32, kind="ExternalInput")
    yT = nc.dram_tensor("yT", [BPC, KT, 128, 1024], BF16,
                        kind="ExternalOutput")
    kTo = nc.dram_tensor("kTo", [BPC, KT, 128, 2080], FP8,
                         kind="ExternalOutput")
    vo = nc.dram_tensor("vo", [BPC, 9, 128, D], BF16, kind="ExternalOutput")
    p8o = nc.dram_tensor("p8o", [BPC, 128, 96], BF16, kind="ExternalOutput")
    dno = nc.dram_tensor("dno", [BPC, 128, 96], F32, kind="ExternalOutput")

    with tile.TileContext(nc) as tc:
        _emit(nc, tc, xT, w_qk, w_v, w_o, b_q, b_v, b_o, yT, kTo, vo, p8o,
              dno)
    nc.compile()
    return nc


def _emit(nc, tc, xT, w_qk, w_v, w_o, b_q, b_v, b_o, yT, kTo, vo, p8o, dno):
    ctx = contextlib.ExitStack()
    with ctx:
        consts = ctx.enter_context(tc.tile_pool(name="consts", bufs=1))
        xpool = ctx.enter_context(tc.tile_pool(name="xpool", bufs=2))
        qpool = ctx.enter_context(tc.tile_pool(name="qpool", bufs=2))
        kpool = ctx.enter_context(tc.tile_pool(name="kpool", bufs=2))
        vpool = ctx.enter_context(tc.tile_pool(name="vpool", bufs=2))
        ptpool = ctx.enter_context(tc.tile_pool(name="ptpool", bufs=11))
        osbpool = ctx.enter_context(tc.tile_pool(name="osbpool", bufs=2))
        otpool = ctx.enter_context(tc.tile_pool(name="otpool", bufs=1))
        recpool = ctx.enter_context(tc.tile_pool(name="recpool", bufs=3))
        p8pool = ctx.enter_context(tc.tile_pool(name="p8pool", bufs=2))
        ytpool = ctx.enter_context(tc.tile_pool(name="ytpool", bufs=2))
        # PSUM: scores 2x[128,1024]=4 banks, pv 1, ds 1, proj 2x[128,512]=2
        bigp = ctx.enter_context(tc.tile_pool(name="bigp", bufs=2,
                                              space="PSUM"))
        pvp = ctx.enter_context(tc.tile_pool(name="pvp", bufs=1,
                                             space="PSUM"))
        dsp = ctx.enter_context(tc.tile_pool(name="dsp", bufs=1,
                                             space="PSUM"))
        projp = ctx.enter_context(tc.tile_pool(name="projp", bufs=2,
                                               space="PSUM"))

        wqk_t = [consts.tile([128, 2 * D], BF16, name=f"wqk{k}")
                 for k in range(KT)]
        wv_t = [consts.tile([128, D], BF16, name=f"wv{k}") for k in range(KT)]
        wo_t = [consts.tile([128, D], BF16, name=f"wo{k}") for k in range(KT)]
        bq_t = [consts.tile([128, 1], F32, name=f"bq{m}") for m in range(KT)]
        bo_t = [consts.tile([128, 1], F32, name=f"bo{m}") for m in range(KT)]
        bv_bc = consts.tile([128, D], F32, name="bvbc")
        ones_t = consts.tile([128, 1], BF16, name="ones")

        xt = {}
        qhi = {}
        khilo = {}
        vt = {}
        osb = {}
        oTt = {}
        p8 = {}
        dnF = {}
        ds = {}

        def alloc_elem(e):
            qhi[e] = [qpool.tile([128, 1024], FP8, tag=f"qhi{m}",
                                 name=f"qhi{e}_{m}") for m in range(KT)]
            khilo[e] = [kpool.tile([128, 2080], FP8, tag=f"kh{m}",
                                   name=f"kh{e}_{m}") for m in range(KT)]
            vt[e] = [vpool.tile([128, D], BF16, tag=f"vt{j}",
                                name=f"vt{e}_{j}") for j in range(9)]
            osb[e] = osbpool.tile([128, 8 * D], BF16, tag="osb",
                                  name=f"osb{e}")
            oTt[e] = otpool.tile([128, KT * 1024], BF16, tag="oT",
                                 name=f"oT{e}")
            p8[e] = p8pool.tile([128, 96], BF16, tag="p8", name=f"p8_{e}")
            dnF[e] = p8pool.tile([128, 96], F32, tag="dn", name=f"dn{e}")
            ds[e] = dsp.tile([128, 512], F32, tag="ds", name=f"ds{e}")

        def load_x(e):
            xt[e] = [xpool.tile([128, L], BF16, tag=f"xt{k}",
                                name=f"xt{e}_{k}") for k in range(KT)]
            for k in range(KT):
                nc.sync.dma_start(out=xt[e][k][:],
                                  in_=xT[e, k * 128:(k + 1) * 128, :])

        def load_x_gen(e):
            load_x(e)
            yield

        def v_unit(e, j):
            jlen = min(128, L - j * 128)
            for (c0, w) in ((0, 512), (512, 256)):
                ps = projp.tile([128, 512], F32, tag="proj",
                                name=f"vps{e}_{j}_{c0}")
                for k in range(KT):
                    nc.tensor.matmul(ps[:jlen, :w],
                                     xt[e][k][:, j * 128:j * 128 + jlen],
                                     wv_t[k][:, c0:c0 + w],
                                     start=(k == 0), stop=(k == KT - 1))
                    if k % 2 == 1:
                        yield
                nc.vector.tensor_tensor(out=vt[e][j][:jlen, c0:c0 + w],
                                        in0=ps[:jlen, :w],
                                        in1=bv_bc[:jlen, c0:c0 + w], op=ADD)
            nc.sync.dma_start(out=vo[e, j][0:jlen, :], in_=vt[e][j][:jlen, :])

        def qk_unit(e, m):
            """m 0..5: Q m-tile -> qhi (fp8, +bias). m 6..11: K m-tile ->
            khilo hi/lo planes (fp8, biasless) + straggler col + export."""
            for c in (0, 1):
                ps = projp.tile([128, 512], F32, tag="proj",
                                name=f"qkps{e}_{m}_{c}")
                for k in range(KT):
                    nc.tensor.matmul(ps[:, :],
                                     wqk_t[k][:, m * 128:(m + 1) * 128],
                                     xt[e][k][:, c * 512:c * 512 + 512],
                                     start=(k == 0), stop=(k == KT - 1))
                    if k % 2 == 1:
                        yield
                if m < KT:
                    nc.vector.tensor_scalar_add(
                        out=_ap(qhi[e][m], 0, 128, c * 512, [[1, 512]]),
                        in0=ps[:, :], scalar1=bq_t[m][:])
                else:
                    kh = khilo[e][m - KT]
                    nc.vector.tensor_copy(
                        out=_ap(kh, 0, 128, c * 512, [[1, 512]]),
                        in_=ps[:, :])
                    nc.vector.tensor_tensor(
                        out=_ap(kh, 0, 128, KLO + c * 512, [[1, 512]]),
                        in0=ps[:, :],
                        in1=_ap(kh, 0, 128, c * 512, [[1, 512]]), op=SUB)
                    yield
            if m >= KT:
                kh = khilo[e][m - KT]
                ps = projp.tile([128, 512], F32, tag="proj",
                                name=f"qksg{e}_{m}")
                for k in range(KT):
                    nc.tensor.matmul(ps[:, 0:1],
                                     wqk_t[k][:, m * 128:(m + 1) * 128],
                                     xt[e][k][:, 1024:1025],
                                     start=(k == 0), stop=(k == KT - 1))
                yield
                nc.vector.tensor_copy(out=_ap(kh, 0, 128, 1024, [[1, 1]]),
                                      in_=ps[:, 0:1])
                nc.vector.tensor_tensor(
                    out=_ap(kh, 0, 128, KLO + 1024, [[1, 1]]),
                    in0=ps[:, 0:1], in1=_ap(kh, 0, 128, 1024, [[1, 1]]),
                    op=SUB)
                nc.sync.dma_start(out=kTo[e, m - KT], in_=kh[:])

        def o_unit(e, m):
            yt = ytpool.tile([128, 1024], BF16, tag="yt", name=f"yt{e}_{m}")
            for c in (0, 1):
                ps = projp.tile([128, 512], F32, tag="proj",
                                name=f"ops{e}_{m}_{c}")
                for k in range(KT):
                    nc.tensor.matmul(
                        ps[:, :], wo_t[k][:, m * 128:(m + 1) * 128],
                        _ap(oTt[e], 0, 128, k * 1024 + c * 512, [[1, 512]]),
                        start=(k == 0), stop=(k == KT - 1))
                    if k % 2 == 1:
                        yield
                nc.vector.tensor_scalar_add(out=yt[:, c * 512:c * 512 + 512],
                                            in0=ps[:, :], scalar1=bo_t[m][:])
            nc.sync.dma_start(out=yT[e, m], in_=yt[:])

        class Fill:
            def __init__(self, gens):
                self.gens = list(gens)

            def pull(self, n=1):
                while n > 0 and self.gens:
                    try:
                        next(self.gens[0])
                        n -= 1
                    except StopIteration:
                        self.gens.pop(0)

            def finish(self, k):
                for gen in self.gens[:k]:
                    for _ in gen:
                        pass
                self.gens = self.gens[k:]

            def flush(self):
                self.finish(len(self.gens))

        def attn_head(e, h, fill):
            mq, poff = h // 2, (h % 2) * 64
            kh, qh = khilo[e][mq], qhi[e][mq]
            # straggler-key scores s8[q, 1] per q-block (N=1 matmuls), then
            # p8 = exp(s8/8) for the host-side rank-12 correction
            for qb in range(8):
                nc.tensor.matmul(_ap(ds[e], 0, 128, 96 + h * 8 + qb, [[1, 1]]),
                                 _ap(qh, poff, 64, qb * 128, [[1, 128]]),
                                 _ap(kh, poff, 64, 1024, [[1, 1]]),
                                 start=True, stop=True)
            nc.scalar.activation(_ap(p8[e], 0, 128, h * 8, [[1, 8]]),
                                 _ap(ds[e], 0, 128, 96 + h * 8, [[1, 8]]),
                                 EXP, bias=0.0, scale=float(SCALE))
            # scores: one DoubleRow matmul per (j, q-half):
            # S^T = Khi^T.Qhi + Klo^T.Qhi
            pts = []
            for j in range(NJ):
                sps = bigp.tile([128, 1024], F32, tag="big",
                                name=f"sps{e}_{h}_{j}")
                for c in (0, 1):
                    nc.tensor.matmul(
                        sps[:, c * 512:c * 512 + 512],
                        _ap(kh, poff, 64, j * 128, [[KLO, 2], [1, 128]]),
                        _ap(qh, poff, 64, c * 512, [[0, 2], [1, 512]]),
                        start=True, stop=True, perf_mode=DR)
                pt = ptpool.tile([128, 1024], BF16, tag="pt",
                                 name=f"pt{e}_{h}_{j}")
                nc.scalar.activation(pt[:, :], sps[:, :], EXP,
                                     bias=0.0, scale=float(SCALE))
                pts.append(pt)
                fill.pull(2)
            # PV (O-orientation) + denominators, qb-outer so each psum
            # accumulation group is a consecutive run of matmuls
            pv = pvp.tile([128, 512], F32, tag="pv", name=f"pv{e}_{h}")
            for qb in range(8):
                for j in range(NJ):
                    nc.tensor.matmul(pv[:, qb * 64:qb * 64 + 64],
                                     pts[j][:, qb * 128:qb * 128 + 128],
                                     vt[e][j][:, h * 64:h * 64 + 64],
                                     start=(j == 0), stop=(j == NJ - 1))
                    nc.tensor.matmul(_ap(ds[e], 0, 128, h * 8 + qb, [[1, 1]]),
                                     pts[j][:, qb * 128:qb * 128 + 128],
                                     ones_t[:, 0:1],
                                     start=(j == 0), stop=(j == NJ - 1))
                fill.pull(1)
            # normalize: rec = 1/(D8 + p8); O_sb[qb, h*64+d] = pv * rec
            nc.vector.tensor_tensor(out=_ap(dnF[e], 0, 128, h * 8, [[1, 8]]),
                                    in0=_ap(ds[e], 0, 128, h * 8, [[1, 8]]),
                                    in1=_ap(p8[e], 0, 128, h * 8, [[1, 8]]),
                                    op=ADD)
            rec = recpool.tile([128, 8], F32, tag="rec", name=f"rec{e}_{h}")
            nc.vector.reciprocal(rec[:, :], _ap(dnF[e], 0, 128, h * 8,
                                                [[1, 8]]))
            nc.vector.tensor_tensor(
                out=_ap(osb[e], 0, 128, h * 64, [[D, 8], [1, 64]]),
                in0=_ap(pv, 0, 128, 0, [[64, 8], [1, 64]]),
                in1=_ap(rec, 0, 128, 0, [[1, 8], [0, 64]]), op=MULT)
            fill.pull(2)

        def transposes(e):
            for qb in range(8):
                nc.sync.dma_start(
                    out=_ap(oTt[e], 0, 128, qb * 128, [[1024, KT], [1, 128]]),
                    in_=_ap(osb[e], 0, 128, qb * D, [[1, D]]),
                    transpose=True)
            nc.sync.dma_start(out=p8o[e], in_=p8[e][:])
            nc.sync.dma_start(out=dno[e], in_=dnF[e][:])

        # ---- schedule ----
        # warm the exp table during the input DMA shadow
        warm = recpool.tile([1, 8], F32, tag="warm", name="warm")
        nc.vector.memset(warm[:1, 0:1], 0.0)
        nc.scalar.activation(warm[:1, 0:1], warm[:1, 0:1], EXP,
                             bias=0.0, scale=1.0)
        nc.vector.memset(ones_t[:], 1.0)
        xt[0] = [xpool.tile([128, L], BF16, tag=f"xt{k}", name=f"xt0_{k}")
                 for k in range(KT)]
        for k in range(KT):
            nc.sync.dma_start(out=xt[0][k][:],
                              in_=xT[0, k * 128:(k + 1) * 128, :])
            nc.sync.dma_start(out=wv_t[k][:], in_=w_v[k * 128:(k + 1) * 128, :])
        bva = b_v[:]
        nc.sync.dma_start(out=bv_bc[:], in_=bass.AP(
            tensor=bva.tensor, offset=bva.offset,
            ap=[[0, 128], list(bva.ap[1])]))
        for k in range(KT):
            nc.sync.dma_start(out=wqk_t[k][:], in_=w_qk[k * 128:(k + 1) * 128, :])
        for m in range(KT):
            nc.sync.dma_start(out=bq_t[m][:], in_=b_q[m * 128:(m + 1) * 128, :])
        for m in range(KT):
            nc.sync.dma_start(out=bo_t[m][:], in_=b_o[m * 128:(m + 1) * 128, :])
        for k in range(KT):
            nc.sync.dma_start(out=wo_t[k][:], in_=w_o[k * 128:(k + 1) * 128, :])

        def run(gen):
            for _ in gen:
                pass

        alloc_elem(0)
        for j in range(9):
            run(v_unit(0, j))
        run(qk_unit(0, 0))
        run(qk_unit(0, 6))

        gens = []
        for m in range(1, KT):
            gens += [qk_unit(0, m), qk_unit(0, KT + m)]
        gens += [load_x_gen(1)]
        alloc_elem(1)
        gens += [v_unit(1, j) for j in range(9)]
        gens += [qk_unit(1, 0), qk_unit(1, KT)]
        fill = Fill(gens)
        for h in range(H):
            if h >= 2 and h % 2 == 0:
                fill.finish(2)
            attn_head(0, h, fill)
        fill.flush()
        transposes(0)

        gens = []
        for m in range(1, KT):
            gens += [qk_unit(1, m), qk_unit(1, KT + m)]
        gens += [o_unit(0, m) for m in range(KT)]
        fill = Fill(gens)
        for h in range(H):
            if h >= 2 and h % 2 == 0:
                fill.finish(2)
            attn_head(1, h, fill)
        fill.flush()
        transposes(1)
        for m in range(KT):
            run(o_unit(1, m))


def _prep_inputs(query, Wqkv, bqkv, Wo, bo):
    Wp = Wqkv.reshape(D, 3, DH, H).transpose(0, 1, 3, 2).reshape(D, 3 * D)
    bp = bqkv.reshape(3, DH, H).transpose(0, 2, 1).reshape(3 * D)
    w_qk = np.ascontiguousarray(Wp[:, :2 * D]).astype(ml_dtypes.bfloat16)
    w_v = np.ascontiguousarray(Wp[:, 2 * D:]).astype(ml_dtypes.bfloat16)
    w_o = np.ascontiguousarray(Wo).astype(ml_dtypes.bfloat16)
    b_q = np.ascontiguousarray(bp[:D]).astype(np.float32).reshape(D, 1)
    b_v = np.ascontiguousarray(bp[2 * D:]).astype(np.float32).reshape(1, D)
    b_o = np.ascontiguousarray(bo).astype(np.float32).reshape(D, 1)

    in_maps = []
    for c in range(N_CORES):
        xc = query[c * BPC:(c + 1) * BPC]
        xTc = np.ascontiguousarray(xc.transpose(0, 2, 1)).astype(
            ml_dtypes.bfloat16)
        in_maps.append(dict(xT=xTc, w_qk=w_qk, w_v=w_v, w_o=w_o,
                            b_q=b_q, b_v=b_v, b_o=b_o))
    return in_maps


def kernel(query, Wqkv, bqkv, Wo, bo):
    query = np.asarray(query, dtype=np.float32)
    Wqkv = np.asarray(Wqkv, dtype=np.float32)
    bqkv = np.asarray(bqkv, dtype=np.float32)
    Wo = np.asarray(Wo, dtype=np.float32)
    bo = np.asarray(bo, dtype=np.float32)

    if "nc" not in _CACHE:
        _CACHE["nc"] = _build()
    nc = _CACHE["nc"]

    in_maps = _prep_inputs(query, Wqkv, bqkv, Wo, bo)
    res = run_bass_kernel_spmd(nc, in_maps, core_ids=list(range(N_CORES)))

    Wp = Wqkv.reshape(D, 3, DH, H).transpose(0, 1, 3, 2).reshape(D, 3 * D)
    bp = bqkv.reshape(3, DH, H).transpose(0, 2, 1).reshape(3 * D)
    out = np.empty((B, L, D), dtype=np.float32)
    for c in range(N_CORES):
        r = res.results[c]
        for e in range(BPC):
            b = c * BPC + e
            # main output: Y^T tiles [6, 128, 1024] -> Y [1024, 768]
            y = np.asarray(r["yT"][e], dtype=np.float32).reshape(
                D, 1024).T.copy()
            # rank-12 straggler-key correction: Y += (p8/D) @ (v_1024 Wo_h)
            p8v = np.asarray(r["p8o"][e], dtype=np.float32)
            dnv = np.asarray(r["dno"][e], dtype=np.float32)
            p8n = (p8v / dnv).reshape(128, H, 8).transpose(2, 0, 1).reshape(
                1024, H)
            v1024 = np.asarray(r["vo"][e, 8, 0], dtype=np.float32)
            w8v = np.einsum("hd,hde->he", v1024.reshape(H, DH),
                            Wo.reshape(H, DH, D))
            y += p8n @ w8v
            out[b, :1024] = y
            # straggler query row: exact host attention from exported K/V
            kt8 = np.asarray(r["kTo"][e], dtype=np.float32)  # [6, 128, 2080]
            kT = (kt8[:, :, :L] + kt8[:, :, KLO:KLO + L]).reshape(D, L)
            vfull = np.asarray(r["vo"][e], dtype=np.float32).reshape(
                9 * 128, D)[:L]
            qrow = query[b, L - 1] @ Wp[:, :D] + bp[:D]
            orow = np.empty(D, dtype=np.float32)
            for h in range(H):
                kh = kT[h * DH:(h + 1) * DH]  # [64, L]
                sh = (qrow[h * DH:(h + 1) * DH] @ kh) * SCALE
                ph = np.exp(sh - sh.max())
                vh = vfull[:, h * DH:(h + 1) * DH]
                orow[h * DH:(h + 1) * DH] = (ph @ vh) / ph.sum()
            out[b, L - 1] = orow @ Wo + bo
    return out
